# revision 37
# baseline (speedup 1.0000x reference)
"""Bass/Trainium2 kernel for nn_GCNNTemporal (GNN message passing over object masks).

Reference computation (B=4 samples, O=8 objects, C=256, HID=128, H=W=64):
  states = relu(conv3x3(concat(feats, mask_o)))          per (sample, object)
  2x:  states_o = relu(conv3x3(concat(states_o, sum_{j!=o} states_j)))
  out_o = sigmoid(conv3x3(concat(feats, states_o)))

Sharding: 2 cores per sample, 4 objects per core. Conv3x3 SAME is 9 shifted
matmuls accumulating in PSUM over a zero-padded flat [128, 66*66] layout.

Pair-dedup of the shared convs (v2):
  - enc: conv(feats) is per-sample; each core convs only its half of the
    pixels (host supplies a haloed half `feats_half`), pairwise AllGather
    rebuilds the full encF (+enc_b folded in at eviction).
  - gcn: agg_o = total - states_o => out_o = relu(conv(s_o, w1-w2)
    + conv(total, w2) + b). ReduceScatter(padded local 4-object sums) hands
    each core its haloed half of total (rank-indexed -> SPMD parity-free);
    each core convs just that half (+gcn_b folded), AllGather rebuilds.
  - readout (Cout=1): 4 objects stacked into K with block-diagonal weights
    (M=4) over 4 PE column strips.
Chunks are row-aligned (7 rows x 66 = 462 cols); evictions write strided
views that skip the 2 pad columns per row, so pads stay zero from a single
init memset (no per-chunk cleanup).
"""
import sys
sys.path.insert(0, '/opt/trn_rl_repo')
import numpy as np

B, O, C, HID, H, W = 4, 8, 256, 128, 64, 64
STEPS = 2
N_CORES = 8

Wp = W + 2                 # padded row width (66)
PADF = (H + 2) * Wp        # 4356 padded flat size
EXT = PADF + 2             # 4358 with +-1 guard elements (data at offset 1)
NINT = H * Wp              # 4224 matmul output columns per image
INT0 = 1 + Wp              # EXT offset of interior (row 1, col 0) = 67
HALF = 32 * Wp             # 2112 cols per half (32 rows)
HEXT = 2 + 34 * Wp         # 2246: 1 guard + 34 haloed padded rows + 1 guard
CHUNKS = [(i * 7 * Wp, 7 * Wp) for i in range(9)] + [(63 * Wp, Wp)]
HSUB = [(j * 7 * Wp, 7 * Wp) for j in range(4)] + [(28 * Wp, 4 * Wp)]
TAPS = [(ky, kx) for ky in range(3) for kx in range(3)]

_PROG_CACHE = {}


def _build_program(repeat=1):
    import concourse.tile as tile
    from concourse import bacc, mybir

    AF = mybir.ActivationFunctionType
    F32 = mybir.dt.float32
    F16 = mybir.dt.float16

    nc = bacc.Bacc("TRN2", target_bir_lowering=False, debug=False,
                   num_devices=N_CORES)

    # ---- DRAM I/O ----
    feats_ap = nc.dram_tensor("feats", [2, 128, EXT], F16, kind="ExternalInput").ap()
    mcols_ap = nc.dram_tensor("mcols", [4, 9, NINT], F16, kind="ExternalInput").ap()
    encw_ap = nc.dram_tensor("encw", [128, 2 * 9 * 128], F16, kind="ExternalInput").ap()
    maskw_ap = nc.dram_tensor("maskw", [128, 128], F16, kind="ExternalInput").ap()
    gcnw12_ap = nc.dram_tensor("gcnw12", [128, 9 * 128], F16, kind="ExternalInput").ap()
    gcnw2_ap = nc.dram_tensor("gcnw2", [128, 9 * 128], F16, kind="ExternalInput").ap()
    row_ap = nc.dram_tensor("row", [128, 6 * 9 * 4], F16, kind="ExternalInput").ap()
    ident_ap = nc.dram_tensor("ident", [128, 128], F16, kind="ExternalInput").ap()
    encb_ap = nc.dram_tensor("encb", [128, 1], F32, kind="ExternalInput").ap()
    gcnb_ap = nc.dram_tensor("gcnb", [128, 1], F32, kind="ExternalInput").ap()
    gcnbT_ap = nc.dram_tensor("gcnbT", [1, 128], F16, kind="ExternalInput").ap()
    rob_ap = nc.dram_tensor("rob", [4, 1], F32, kind="ExternalInput").ap()
    out_ap = nc.dram_tensor("out", [4, H * W], F32, kind="ExternalOutput").ap()

    with tile.TileContext(nc) as tc:
        with tc.tile_pool(name="persist", bufs=1) as pp, \
             tc.tile_pool(name="psum", bufs=8, space="PSUM") as psp, \
             tc.tile_pool(name="dram", bufs=1, space="DRAM") as dp:

            # ---- persistent SBUF ----
            sts = [pp.tile([128, EXT], F16, tag=f"st{i}", name=f"st{i}")
                   for i in range(5)]
            feats_sb = [pp.tile([128, EXT], F16, tag=f"feat{k}", name=f"feat{k}")
                        for k in range(2)]
            shared_sb = pp.tile([128, NINT], F16, tag="shared")   # encF / gcn total-conv
            tot_ext = pp.tile([128, HEXT], F16, tag="totext")
            lsum_sb = pp.tile([128, NINT], F16, tag="lsum")
            hv_sb = pp.tile([128, HALF], F16, tag="hv")
            gcnw12_sb = pp.tile([128, 9 * 128], F16, tag="gw12")
            gcnw2_sb = pp.tile([128, 9 * 128], F16, tag="gw2")
            row_sb = pp.tile([128, 6 * 9 * 4], F16, tag="row")
            encb_sb = pp.tile([128, 1], F32, tag="encb")
            gcnb_sb = pp.tile([128, 1], F32, tag="gcnb")
            rob_sb = pp.tile([4, 1], F32, tag="rob")
            ident_sb = pp.tile([128, 128], F16, tag="ident")
            gcnbT_sb = pp.tile([1, 128], F16, tag="gcnbT")
            ones_sb = pp.tile([1, 7 * Wp], F16, tag="ones")
            zrow_sb = pp.tile([128, Wp], F16, tag="zrow")
            parks = [pp.tile([128, NINT], F16, tag=f"park{i}", name=f"park{i}")
                     for i in range(4)]

            # ---- DRAM collective buffers ----
            ccw_in = dp.tile([1, 1], F32, tag="ccwin")
            ccw_out = dp.tile([1, 1], F32, tag="ccwout")
            ar_ins = [dp.tile([2, 128, HALF + 2 * Wp], F16, tag=f"arin{s}",
                              name=f"arin{s}") for s in range(STEPS)]
            rs_outs = [dp.tile([128, HALF + 2 * Wp], F16, tag=f"rsout{s}",
                               name=f"rsout{s}")
                       for s in range(STEPS)]
            HQ = HALF // 2  # 1056 = 16 rows
            ag_ins = [[dp.tile([128, HQ], F16, tag=f"agin{s}{h}",
                               name=f"agin{s}{h}") for h in range(2)]
                      for s in range(STEPS)]
            ag_outs = [[dp.tile([2, 128, HQ], F16, tag=f"agout{s}{h}",
                                name=f"agout{s}{h}") for h in range(2)]
                       for s in range(STEPS)]
            GROUPS = [[0, 1], [2, 3], [4, 5], [6, 7]]

            # tiny warm-up collective absorbs CC-stream init latency
            warm_sb = pp.tile([1, 1], F32, tag="warm")
            nc.vector.memset(warm_sb[:], 0.0)
            nc.sync.dma_start(out=ccw_in[:], in_=warm_sb[:])
            nc.gpsimd.collective_compute(
                "AllReduce", mybir.AluOpType.add,
                replica_groups=GROUPS,
                ins=[ccw_in.opt()], outs=[ccw_out.opt()])

            # zero guard/pad regions once (never rewritten afterwards)
            nc.vector.memset(ones_sb[:], 1.0)
            nc.vector.memset(zrow_sb[:], 0.0)
            for t_ in sts:
                nc.vector.memset(t_[:, 0:INT0], 0.0)
                nc.vector.memset(t_[:, INT0 + NINT:EXT], 0.0)
                nc.vector.memset(t_[:, INT0:INT0 + NINT:Wp], 0.0)
                nc.vector.memset(t_[:, INT0 + Wp - 1:INT0 + NINT:Wp], 0.0)
            nc.vector.memset(tot_ext[:, 0:1], 0.0)
            nc.vector.memset(tot_ext[:, HEXT - 1:HEXT], 0.0)
            # payload shard pad rows (row above shard0 / below shard1 of the
            # padded 66-row image): zero once (scalar queue: off critical path)
            for s in range(STEPS):
                nc.scalar.dma_start(out=ar_ins[s][0, :, 0:Wp], in_=zrow_sb[:])
                nc.scalar.dma_start(out=ar_ins[s][1, :, HALF + Wp:HALF + 2 * Wp],
                                    in_=zrow_sb[:])

            def iview(t, nb, nw):
                """Strided interior view [128, nr, 64] skipping pad columns."""
                nr = nw // Wp
                return t[:, INT0 + nb:INT0 + nb + nw] \
                    .rearrange("p (r u) -> p r u", u=Wp)[:, :, 1:1 + W]

            def pview(ps, nw):
                nr = nw // Wp
                return ps[:, 0:nw].rearrange("p (r u) -> p r u", u=Wp)[:, :, 1:1 + W]

            def conv_mms(ps, w_sb, w_idx, src, nb, nw, first, last, m=128):
                """9 accumulating tap matmuls into psum ps."""
                for t, (ky, kx) in enumerate(TAPS):
                    off = ky * Wp + kx
                    nc.tensor.matmul(
                        ps[:], w_sb[:, (w_idx * 9 + t) * m:(w_idx * 9 + t + 1) * m],
                        src[:, off + nb: off + nb + nw],
                        start=(first and t == 0), stop=(last and t == 8))

            def relu_evict(dst_t, ps, nb, nw, eng):
                """dst = relu(psum), strided (pads skipped), biasless."""
                dv, sv = iview(dst_t, nb, nw), pview(ps, nw)
                if eng == 's':
                    nc.scalar.activation(dv, sv, AF.Relu)
                elif eng == 'v':
                    nc.vector.tensor_scalar_max(dv, sv, 0.0)
                else:
                    nc.gpsimd.tensor_scalar_max(dv, sv, 0.0)

            def accum_lsum(j, sts_now, nb, nw):
                """Fold object j's chunk into the running fp16 local sum."""
                if j == 0:
                    return
                acc = lsum_sb[:, nb:nb + nw]
                s = sts_now[j][:, INT0 + nb:INT0 + nb + nw]
                if j == 1:
                    nc.vector.tensor_add(acc, sts_now[0][:, INT0 + nb:INT0 + nb + nw], s)
                else:
                    nc.vector.tensor_add(acc, acc, s)

            def payload_dmas(s):
                """lsum -> padded overlapping shards of ar_ins[s]."""
                ar = ar_ins[s]
                # shard0 = padded rows 0..33: pad row (pre-zeroed) + interior
                # rows 0..32; shard1 = interior rows 31..63 + pad row.
                nc.sync.dma_start(out=ar[0, :, Wp:HALF + 2 * Wp],
                                  in_=lsum_sb[:, 0:HALF + Wp])
                nc.sync.dma_start(out=ar[1, :, 0:HALF + Wp],
                                  in_=lsum_sb[:, HALF - Wp:NINT])

            cur = [sts[0], sts[1], sts[2], sts[3]]
            spare = sts[4]

            for _rep in range(repeat):
                with tc.tile_pool(name="encpool", bufs=1) as ep:
                    encw_sb = ep.tile([128, 2 * 9 * 128], F16, tag="encw")
                    maskw_sb = ep.tile([128, 128], F16, tag="maskw")
                    mask_sb = ep.tile([128, NINT], F16, tag="maskcols")
                    # critical-path DMAs first: enc weights + feats, sliced so
                    # chunk 0 can start after ~1MB instead of ~3MB
                    fcuts = [0, 726, 2444, EXT]
                    nc.sync.dma_start(out=encw_sb[:, 0:1152], in_=encw_ap[:, 0:1152])
                    nc.sync.dma_start(out=feats_sb[0][:, 0:726],
                                      in_=feats_ap[0, :, 0:726])
                    nc.sync.dma_start(out=feats_sb[1][:, 0:726],
                                      in_=feats_ap[1, :, 0:726])
                    nc.sync.dma_start(out=encw_sb[:, 1152:2304],
                                      in_=encw_ap[:, 1152:2304])
                    for a, b in zip(fcuts[1:], fcuts[2:]):
                        nc.sync.dma_start(out=feats_sb[0][:, a:b],
                                          in_=feats_ap[0, :, a:b])
                        nc.sync.dma_start(out=feats_sb[1][:, a:b],
                                          in_=feats_ap[1, :, a:b])
                    # bulk / non-critical DMAs go on the scalar-issued queue so
                    # the sync queue stays clear for the collective chain
                    nc.scalar.dma_start(out=encb_sb[:], in_=encb_ap[:])
                    nc.scalar.dma_start(out=maskw_sb[:], in_=maskw_ap[:])
                    for j in range(4):
                        nc.scalar.dma_start(out=mask_sb[32 * j:32 * j + 9, :],
                                            in_=mcols_ap[j])
                    if _rep == 0:
                        nc.scalar.dma_start(out=ident_sb[:], in_=ident_ap[:])
                        nc.scalar.dma_start(out=gcnb_sb[:], in_=gcnb_ap[:])
                        nc.scalar.dma_start(out=gcnbT_sb[:], in_=gcnbT_ap[:])
                        nc.scalar.dma_start(out=rob_sb[:], in_=rob_ap[:])
                        nc.scalar.dma_start(out=gcnw12_sb[:], in_=gcnw12_ap[:])
                        nc.scalar.dma_start(out=gcnw2_sb[:], in_=gcnw2_ap[:])
                        nc.scalar.dma_start(out=row_sb[:], in_=row_ap[:])

                    # ---- enc shared feats conv (full width; the CC stream
                    # takes ~50us to come up, so a dedup AllGather here would
                    # stall longer than the 9N of duplicated PE it saves) ----
                    for nb, nw in CHUNKS:
                        ps = psp.tile([128, nw], F32, tag="cps")
                        for kt in range(2):
                            conv_mms(ps, encw_sb, kt, feats_sb[kt], nb, nw,
                                     first=(kt == 0), last=(kt == 1))
                        nc.scalar.activation(shared_sb[:, nb:nb + nw], ps[:],
                                             AF.Identity, bias=encb_sb[:])

                    # ---- per-object: mask conv + identity(shared) + relu ----
                    # (gpsimd cannot read PSUM -> scalar/vector only here)
                    ENG = ['s', 'v', 's', 'v']
                    for nb, nw in CHUNKS:
                        pss = [psp.tile([128, nw], F32, tag="cps", name=f"mps{j}")
                               for j in range(4)]
                        for j in range(4):
                            nc.tensor.matmul(pss[j][:], maskw_sb[32 * j:32 * j + 9, :],
                                             mask_sb[32 * j:32 * j + 9, nb:nb + nw],
                                             start=True, stop=False,
                                             tile_position=(32 * j, 0))
                        for j in range(4):
                            nc.tensor.matmul(pss[j][:], ident_sb[:],
                                             shared_sb[:, nb:nb + nw],
                                             start=False, stop=True)
                        for j in range(4):
                            relu_evict(cur[j], pss[j], nb, nw, ENG[j])
                            accum_lsum(j, cur, nb, nw)
                    payload_dmas(0)

                # ================= GCN x2 =================
                with tc.tile_pool(name="gcnpool", bufs=1) as gp:
                    for step in range(STEPS):
                        ar, rs_out = ar_ins[step], rs_outs[step]
                        ag_in, ag_out = ag_ins[step], ag_outs[step]
                        nc.gpsimd.collective_compute(
                            "ReduceScatter", mybir.AluOpType.add,
                            replica_groups=GROUPS,
                            ins=[ar.opt()], outs=[rs_out.opt()])
                        nc.sync.dma_start(out=tot_ext[:, 1:HEXT - 1], in_=rs_out[:])

                        # park objects 1,2 while the ReduceScatter flies
                        # (RS kicks right at step start; 18N of independent PE
                        # work covers its ~13us plus the late payload)
                        for j in (1, 2):
                            for ci, (nb, nw) in enumerate(CHUNKS):
                                ps = psp.tile([128, nw], F32, tag="cps")
                                conv_mms(ps, gcnw12_sb, 0, cur[j], nb, nw, True, True)
                                if (ci + j) % 2 == 0:
                                    nc.scalar.activation(parks[j][:, nb:nb + nw],
                                                         ps[:], AF.Copy)
                                else:
                                    nc.vector.tensor_copy(parks[j][:, nb:nb + nw],
                                                          ps[:])
                        # half-conv of total (+gcn_b via a K=1 ones-row matmul
                        # so the eviction is a plain copy on the vector engine,
                        # keeping the scalar queue off the AllGather chain)
                        for cnb, cnw in HSUB:
                            ps = psp.tile([128, cnw], F32, tag="cps")
                            for t, (ky, kx) in enumerate(TAPS):
                                off = ky * Wp + kx
                                nc.tensor.matmul(
                                    ps[:], gcnw2_sb[:, t * 128:(t + 1) * 128],
                                    tot_ext[:, off + cnb: off + cnb + cnw],
                                    start=(t == 0), stop=False)
                            nc.tensor.matmul(ps[:], gcnbT_sb[:],
                                             ones_sb[:, 0:cnw],
                                             start=False, stop=True)
                            nc.vector.tensor_copy(hv_sb[:, cnb:cnb + cnw], ps[:])
                        # AllGather in two halves so the first quarter-rows of
                        # shared land earlier and finalize starts sooner
                        for h in range(2):
                            nc.sync.dma_start(out=ag_in[h][:],
                                              in_=hv_sb[:, h * HQ:(h + 1) * HQ])
                            nc.gpsimd.collective_compute(
                                "AllGather", mybir.AluOpType.bypass,
                                replica_groups=GROUPS,
                                ins=[ag_in[h].opt()], outs=[ag_out[h].opt()])
                        # park object 3 while the AllGather flies
                        for ci, (nb, nw) in enumerate(CHUNKS):
                            ps = psp.tile([128, nw], F32, tag="cps")
                            conv_mms(ps, gcnw12_sb, 0, cur[3], nb, nw, True, True)
                            if (ci + 1) % 2 == 0:
                                nc.scalar.activation(parks[3][:, nb:nb + nw],
                                                     ps[:], AF.Copy)
                            else:
                                nc.vector.tensor_copy(parks[3][:, nb:nb + nw], ps[:])
                        # ag half h carries local rows [16h,16h+16) of both
                        # cores: even core -> global rows 16h.., odd -> 32+16h..
                        for h in range(2):
                            nc.sync.dma_start(out=shared_sb[:, h * HQ:(h + 1) * HQ],
                                              in_=ag_out[h][0])
                            nc.sync.dma_start(
                                out=shared_sb[:, HALF + h * HQ:HALF + (h + 1) * HQ],
                                in_=ag_out[h][1])

                        dsts = [spare, cur[0], cur[1], cur[2]]
                        # object 0 direct once shared is in SBUF: conv +
                        # identity(shared) in PSUM, relu straight out — no
                        # park copy, no add in the congested finalize window
                        for ci, (nb, nw) in enumerate(CHUNKS):
                            ps = psp.tile([128, nw], F32, tag="cps")
                            conv_mms(ps, gcnw12_sb, 0, cur[0], nb, nw, True, False)
                            nc.tensor.matmul(ps[:], ident_sb[:],
                                             shared_sb[:, nb:nb + nw],
                                             start=False, stop=True)
                            relu_evict(dsts[0], ps, nb, nw, 's' if ci % 2 else 'v')

                        # finalize objects 1..3: states_new = relu(park + shared)
                        ENG = [None, 'v', 's', 'v']
                        for nb, nw in CHUNKS:
                            for j in (1, 2, 3):
                                nc.vector.tensor_add(parks[j][:, nb:nb + nw],
                                                     parks[j][:, nb:nb + nw],
                                                     shared_sb[:, nb:nb + nw])
                                dv = iview(dsts[j], nb, nw)
                                sv = parks[j][:, nb:nb + nw] \
                                    .rearrange("p (r u) -> p r u", u=Wp)[:, :, 1:1 + W]
                                if ENG[j] == 's':
                                    nc.scalar.activation(dv, sv, AF.Relu)
                                else:
                                    nc.vector.tensor_scalar_max(dv, sv, 0.0)
                                if step == 0:
                                    accum_lsum(j, dsts, nb, nw)
                        if step == 0:
                            payload_dmas(1)
                        new_spare = cur[3]
                        cur = [dsts[0], dsts[1], dsts[2], dsts[3]]
                        spare = new_spare

                # ================= READOUT =================
                # M=4: 54 (ktile, tap) accumulating matmuls split over 4 PE
                # column strips, issued round-robin for strip concurrency.
                with tc.tile_pool(name="ropool", bufs=1) as rp:
                    out_sb = rp.tile([4, NINT], F32, tag="outsb")
                    strips = [
                        [(4, t) for t in range(9)] + [(2, t) for t in range(5)],
                        [(5, t) for t in range(9)] + [(2, t) for t in range(5, 9)]
                        + [(3, 0)],
                        [(0, t) for t in range(4)] + [(1, t) for t in range(5)]
                        + [(3, t) for t in range(1, 5)],
                        [(0, t) for t in range(4, 9)] + [(1, t) for t in range(5, 9)]
                        + [(3, t) for t in range(5, 9)],
                    ]
                    ov = out_ap.rearrange("o (y x) -> o y x", x=W)
                    iv = out_sb[:].rearrange("o (y u) -> o y u", u=Wp)[:, :, 1:1 + W]
                    for nb, nw in CHUNKS:
                        pss = [psp.tile([128, nw], F32, tag="cps", name=f"rops{g}")
                               for g in range(4)]
                        for i in range(14):
                            for g, chain in enumerate(strips):
                                if i >= len(chain):
                                    continue
                                k, t = chain[i]
                                src = cur[k] if k < 4 else feats_sb[k - 4]
                                ky, kx = TAPS[t]
                                off = ky * Wp + kx
                                nc.tensor.matmul(
                                    pss[g][32 * g:32 * g + 4, :],
                                    row_sb[:, (k * 9 + t) * 4:(k * 9 + t + 1) * 4],
                                    src[:, off + nb: off + nb + nw],
                                    start=(i == 0), stop=(i == len(chain) - 1),
                                    tile_position=(0, 32 * g))
                        o = out_sb[:, nb:nb + nw]
                        nc.vector.tensor_copy(o, pss[0][0:4, :])
                        nc.vector.tensor_add(o, o, pss[1][32:36, :])
                        nc.vector.tensor_add(o, o, pss[2][64:68, :])
                        nc.vector.tensor_add(o, o, pss[3][96:100, :])
                        nc.scalar.activation(o, o, AF.Sigmoid, bias=rob_sb[:])
                        r0, nr = nb // Wp, nw // Wp
                        nc.scalar.dma_start(out=ov[:, r0:r0 + nr],
                                            in_=iv[:, r0:r0 + nr])

    nc.compile()
    return nc


def _host_prep(inputs):
    """Per-core input maps: shard + pad + im2col + weight lhsT layouts."""
    feats = np.asarray(inputs["batch_node_feats"], np.float32)
    masks = np.asarray(inputs["batch_previous_masks"], np.float32)
    enc_w = np.asarray(inputs["enc_w"], np.float32)
    enc_b = np.asarray(inputs["enc_b"], np.float32)
    gcn_w = np.asarray(inputs["gcn_w"], np.float32)
    gcn_b = np.asarray(inputs["gcn_b"], np.float32)
    ro_w = np.asarray(inputs["ro_w"], np.float32)
    ro_b = np.asarray(inputs["ro_b"], np.float32)

    # ---- weights (shared across cores) ----
    # enc feats part: lhsT [128cin_part, ktile, tap, cout]
    encw = enc_w[:, :C].transpose(2, 3, 1, 0).reshape(9, 2, 128, HID) \
        .transpose(2, 1, 0, 3).reshape(128, 2 * 9 * HID).copy()
    # enc mask channel: K=9 lhsT replicated at partitions {0,32,64,96}
    mvec = enc_w[:, C].transpose(1, 2, 0).reshape(9, HID)  # [tap, cout]
    maskw = np.zeros((128, 128), np.float32)
    for j in range(4):
        maskw[32 * j:32 * j + 9] = mvec
    w1 = gcn_w[:, :HID]
    w2 = gcn_w[:, HID:]
    gcnw12 = (w1 - w2).transpose(2, 3, 1, 0).reshape(9, 128, 128) \
        .transpose(1, 0, 2).reshape(128, 9 * 128).copy()
    gcnw2 = w2.transpose(2, 3, 1, 0).reshape(9, 128, 128) \
        .transpose(1, 0, 2).reshape(128, 9 * 128).copy()
    # readout: [6, 9, 128, 4]
    row = np.zeros((6, 9, 128, 4), np.float32)
    rs = ro_w[0, C:].transpose(1, 2, 0).reshape(9, HID)   # states part [tap, cin]
    for k in range(4):
        row[k, :, :, k] = rs
    for k, sl in ((4, ro_w[0, :128]), (5, ro_w[0, 128:256])):
        row[k] = sl.transpose(1, 2, 0).reshape(9, 128)[:, :, None]
    encb = enc_b.reshape(128, 1).astype(np.float32)
    gcnb = gcn_b.reshape(128, 1).astype(np.float32)
    rob = np.broadcast_to(ro_b.reshape(1, 1), (4, 1)).astype(np.float32).copy()

    in_maps = []
    for c in range(N_CORES):
        s, half = c // 2, c % 2
        # feats: pad to [C, 66, 66], flat ext [C, 4358] at offset 1
        fp = np.zeros((C, H + 2, Wp), np.float32)
        fp[:, 1:H + 1, 1:W + 1] = feats[s]
        fpflat = fp.reshape(C, PADF)
        fe = np.zeros((C, EXT), np.float32)
        fe[:, 1:1 + PADF] = fpflat
        # masks im2col: [4, 9, NINT]
        mc = np.zeros((4, 9, NINT), np.float32)
        for j in range(4):
            mp = np.zeros((H + 2, Wp), np.float32)
            mp[1:H + 1, 1:W + 1] = masks[s, 4 * half + j]
            mf = np.zeros(EXT, np.float32)
            mf[1:1 + PADF] = mp.reshape(PADF)
            for t, (ky, kx) in enumerate(TAPS):
                off = ky * Wp + kx
                mc[j, t] = mf[off:off + NINT]
        in_maps.append({
            "feats": fe.reshape(2, 128, EXT).astype(np.float16),
            "mcols": mc.astype(np.float16),
            "encw": encw.astype(np.float16), "maskw": maskw.astype(np.float16),
            "gcnw12": gcnw12.astype(np.float16), "gcnw2": gcnw2.astype(np.float16),
            "row": row.transpose(2, 0, 1, 3).reshape(128, 6 * 9 * 4).astype(np.float16),
            "ident": np.eye(128, dtype=np.float16),
            "encb": encb, "gcnb": gcnb, "rob": rob,
            "gcnbT": gcnb.reshape(1, 128).astype(np.float16),
        })
    return in_maps


def _run(inputs, repeat=1):
    from concourse.bass_utils import run_bass_kernel_spmd
    if repeat not in _PROG_CACHE:
        _PROG_CACHE[repeat] = _build_program(repeat)
    nc = _PROG_CACHE[repeat]
    in_maps = _host_prep(inputs)
    r = run_bass_kernel_spmd(nc, in_maps, list(range(N_CORES)))
    out = np.zeros((B, O, H, W), np.float32)
    for c in range(N_CORES):
        s, half = c // 2, c % 2
        out[s, 4 * half:4 * half + 4] = r.results[c]["out"].reshape(4, H, W)
    return out


def kernel(**inputs) -> np.ndarray:
    return _run(inputs, repeat=1)


# revision 39
# speedup vs baseline: 1.0852x; 1.0852x over previous
"""Bass/Trainium2 kernel for nn_GCNNTemporal (GNN message passing over object masks).

Reference computation (B=4 samples, O=8 objects, C=256, HID=128, H=W=64):
  states = relu(conv3x3(concat(feats, mask_o)))          per (sample, object)
  2x:  states_o = relu(conv3x3(concat(states_o, sum_{j!=o} states_j)))
  out_o = sigmoid(conv3x3(concat(feats, states_o)))

Sharding: 2 cores per sample, 4 objects per core. Conv3x3 SAME is 9 shifted
matmuls accumulating in PSUM over a zero-padded flat [128, 66*66] layout.

Pair-dedup of the shared convs (v2):
  - enc: conv(feats) is per-sample; each core convs only its half of the
    pixels (host supplies a haloed half `feats_half`), pairwise AllGather
    rebuilds the full encF (+enc_b folded in at eviction).
  - gcn: agg_o = total - states_o => out_o = relu(conv(s_o, w1-w2)
    + conv(total, w2) + b). ReduceScatter(padded local 4-object sums) hands
    each core its haloed half of total (rank-indexed -> SPMD parity-free);
    each core convs just that half (+gcn_b folded), AllGather rebuilds.
  - readout (Cout=1): 4 objects stacked into K with block-diagonal weights
    (M=4) over 4 PE column strips.
Chunks are row-aligned (7 rows x 66 = 462 cols); evictions write strided
views that skip the 2 pad columns per row, so pads stay zero from a single
init memset (no per-chunk cleanup).
"""
import sys
sys.path.insert(0, '/opt/trn_rl_repo')
import numpy as np

B, O, C, HID, H, W = 4, 8, 256, 128, 64, 64
STEPS = 2
N_CORES = 8

Wp = W + 2                 # padded row width (66)
PADF = (H + 2) * Wp        # 4356 padded flat size
EXT = PADF + 2             # 4358 with +-1 guard elements (data at offset 1)
NINT = H * Wp              # 4224 matmul output columns per image
INT0 = 1 + Wp              # EXT offset of interior (row 1, col 0) = 67
HALF = 32 * Wp             # 2112 cols per half (32 rows)
HEXT = 2 + 34 * Wp         # 2246: 1 guard + 34 haloed padded rows + 1 guard
CHUNKS = [(i * 7 * Wp, 7 * Wp) for i in range(9)] + [(63 * Wp, Wp)]
HSUB = [(j * 7 * Wp, 7 * Wp) for j in range(4)] + [(28 * Wp, 4 * Wp)]
TAPS = [(ky, kx) for ky in range(3) for kx in range(3)]

_PROG_CACHE = {}


def _build_program(repeat=1):
    import concourse.tile as tile
    from concourse import bacc, mybir

    AF = mybir.ActivationFunctionType
    F32 = mybir.dt.float32
    F16 = mybir.dt.float16

    nc = bacc.Bacc("TRN2", target_bir_lowering=False, debug=False,
                   num_devices=N_CORES)

    # ---- DRAM I/O ----
    feats_ap = nc.dram_tensor("feats", [2, 128, EXT], F16, kind="ExternalInput").ap()
    mcols_ap = nc.dram_tensor("mcols", [4, 9, NINT], F16, kind="ExternalInput").ap()
    encw_ap = nc.dram_tensor("encw", [128, 2 * 9 * 128], F16, kind="ExternalInput").ap()
    maskw_ap = nc.dram_tensor("maskw", [128, 128], F16, kind="ExternalInput").ap()
    gcnw12_ap = nc.dram_tensor("gcnw12", [128, 9 * 128], F16, kind="ExternalInput").ap()
    gcnw2_ap = nc.dram_tensor("gcnw2", [128, 9 * 128], F16, kind="ExternalInput").ap()
    row_ap = nc.dram_tensor("row", [128, 6 * 9 * 4], F16, kind="ExternalInput").ap()
    ident_ap = nc.dram_tensor("ident", [128, 128], F16, kind="ExternalInput").ap()
    encb_ap = nc.dram_tensor("encb", [128, 1], F32, kind="ExternalInput").ap()
    gcnb_ap = nc.dram_tensor("gcnb", [128, 1], F32, kind="ExternalInput").ap()
    gcnbT_ap = nc.dram_tensor("gcnbT", [1, 128], F16, kind="ExternalInput").ap()
    rob_ap = nc.dram_tensor("rob", [4, 1], F32, kind="ExternalInput").ap()
    out_ap = nc.dram_tensor("out", [4, H * W], F32, kind="ExternalOutput").ap()

    with tile.TileContext(nc) as tc:
        with tc.tile_pool(name="persist", bufs=1) as pp, \
             tc.tile_pool(name="psum", bufs=8, space="PSUM") as psp, \
             tc.tile_pool(name="dram", bufs=1, space="DRAM") as dp:

            # ---- persistent SBUF ----
            sts = [pp.tile([128, EXT], F16, tag=f"st{i}", name=f"st{i}")
                   for i in range(5)]
            feats_sb = [pp.tile([128, EXT], F16, tag=f"feat{k}", name=f"feat{k}")
                        for k in range(2)]
            shared_sb = pp.tile([128, NINT], F16, tag="shared")   # encF / gcn total-conv
            tot_ext = pp.tile([128, HEXT], F16, tag="totext")
            lsum_sb = pp.tile([128, NINT], F16, tag="lsum")
            hv_sb = pp.tile([128, HALF], F16, tag="hv")
            gcnw12_sb = pp.tile([128, 9 * 128], F16, tag="gw12")
            gcnw2_sb = pp.tile([128, 9 * 128], F16, tag="gw2")
            row_sb = pp.tile([128, 6 * 9 * 4], F16, tag="row")
            encb_sb = pp.tile([128, 1], F32, tag="encb")
            gcnb_sb = pp.tile([128, 1], F32, tag="gcnb")
            rob_sb = pp.tile([4, 1], F32, tag="rob")
            ident_sb = pp.tile([128, 128], F16, tag="ident")
            gcnbT_sb = pp.tile([1, 128], F16, tag="gcnbT")
            ones_sb = pp.tile([1, 7 * Wp], F16, tag="ones")
            zrow_sb = pp.tile([128, Wp], F16, tag="zrow")
            parks = [pp.tile([128, NINT], F16, tag=f"park{i}", name=f"park{i}")
                     for i in range(4)]

            # ---- DRAM collective buffers ----
            ccw_in = dp.tile([1, 1], F32, tag="ccwin")
            ccw_out = dp.tile([1, 1], F32, tag="ccwout")
            ar_ins = [dp.tile([2, 128, HALF + 2 * Wp], F16, tag=f"arin{s}",
                              name=f"arin{s}") for s in range(STEPS)]
            rs_outs = [dp.tile([128, HALF + 2 * Wp], F16, tag=f"rsout{s}",
                               name=f"rsout{s}")
                       for s in range(STEPS)]
            HQ = HALF // 2  # 1056 = 16 rows
            ag_ins = [[dp.tile([128, HQ], F16, tag=f"agin{s}{h}",
                               name=f"agin{s}{h}") for h in range(2)]
                      for s in range(STEPS)]
            ag_outs = [[dp.tile([2, 128, HQ], F16, tag=f"agout{s}{h}",
                                name=f"agout{s}{h}") for h in range(2)]
                       for s in range(STEPS)]
            GROUPS = [[0, 1], [2, 3], [4, 5], [6, 7]]

            # tiny warm-up collective absorbs CC-stream init latency
            warm_sb = pp.tile([1, 1], F32, tag="warm")
            nc.vector.memset(warm_sb[:], 0.0)
            nc.sync.dma_start(out=ccw_in[:], in_=warm_sb[:])
            nc.gpsimd.collective_compute(
                "AllReduce", mybir.AluOpType.add,
                replica_groups=GROUPS,
                ins=[ccw_in.opt()], outs=[ccw_out.opt()])

            # zero guard/pad regions once (never rewritten afterwards)
            nc.vector.memset(ones_sb[:], 1.0)
            nc.vector.memset(zrow_sb[:], 0.0)
            for t_ in sts:
                nc.vector.memset(t_[:, 0:INT0], 0.0)
                nc.vector.memset(t_[:, INT0 + NINT:EXT], 0.0)
                nc.vector.memset(t_[:, INT0:INT0 + NINT:Wp], 0.0)
                nc.vector.memset(t_[:, INT0 + Wp - 1:INT0 + NINT:Wp], 0.0)
            nc.vector.memset(tot_ext[:, 0:1], 0.0)
            nc.vector.memset(tot_ext[:, HEXT - 1:HEXT], 0.0)
            # payload shard pad rows (row above shard0 / below shard1 of the
            # padded 66-row image): zero once (scalar queue: off critical path)
            for s in range(STEPS):
                nc.scalar.dma_start(out=ar_ins[s][0, :, 0:Wp], in_=zrow_sb[:])
                nc.scalar.dma_start(out=ar_ins[s][1, :, HALF + Wp:HALF + 2 * Wp],
                                    in_=zrow_sb[:])

            def iview(t, nb, nw):
                """Strided interior view [128, nr, 64] skipping pad columns."""
                nr = nw // Wp
                return t[:, INT0 + nb:INT0 + nb + nw] \
                    .rearrange("p (r u) -> p r u", u=Wp)[:, :, 1:1 + W]

            def pview(ps, nw):
                nr = nw // Wp
                return ps[:, 0:nw].rearrange("p (r u) -> p r u", u=Wp)[:, :, 1:1 + W]

            def conv_mms(ps, w_sb, w_idx, src, nb, nw, first, last, m=128):
                """9 accumulating tap matmuls into psum ps."""
                for t, (ky, kx) in enumerate(TAPS):
                    off = ky * Wp + kx
                    nc.tensor.matmul(
                        ps[:], w_sb[:, (w_idx * 9 + t) * m:(w_idx * 9 + t + 1) * m],
                        src[:, off + nb: off + nb + nw],
                        start=(first and t == 0), stop=(last and t == 8))

            def relu_evict(dst_t, ps, nb, nw, eng):
                """dst = relu(psum), strided (pads skipped), biasless."""
                dv, sv = iview(dst_t, nb, nw), pview(ps, nw)
                if eng == 's':
                    nc.scalar.activation(dv, sv, AF.Relu)
                elif eng == 'v':
                    nc.vector.tensor_scalar_max(dv, sv, 0.0)
                else:
                    nc.gpsimd.tensor_scalar_max(dv, sv, 0.0)

            def accum_lsum(j, sts_now, nb, nw):
                """Fold object j's chunk into the running fp16 local sum."""
                if j == 0:
                    return
                acc = lsum_sb[:, nb:nb + nw]
                s = sts_now[j][:, INT0 + nb:INT0 + nb + nw]
                if j == 1:
                    nc.vector.tensor_add(acc, sts_now[0][:, INT0 + nb:INT0 + nb + nw], s)
                else:
                    nc.vector.tensor_add(acc, acc, s)

            def payload_dmas(s):
                """lsum -> padded overlapping shards of ar_ins[s]."""
                ar = ar_ins[s]
                # shard0 = padded rows 0..33: pad row (pre-zeroed) + interior
                # rows 0..32; shard1 = interior rows 31..63 + pad row.
                nc.sync.dma_start(out=ar[0, :, Wp:HALF + 2 * Wp],
                                  in_=lsum_sb[:, 0:HALF + Wp])
                nc.sync.dma_start(out=ar[1, :, 0:HALF + Wp],
                                  in_=lsum_sb[:, HALF - Wp:NINT])

            cur = [sts[0], sts[1], sts[2], sts[3]]
            spare = sts[4]

            for _rep in range(repeat):
                with tc.tile_pool(name="encpool", bufs=1) as ep:
                    encw_sb = ep.tile([128, 2 * 9 * 128], F16, tag="encw")
                    maskw_sb = ep.tile([128, 128], F16, tag="maskw")
                    mask_sb = ep.tile([128, NINT], F16, tag="maskcols")
                    # critical-path DMAs first: enc weights + feats, sliced so
                    # chunk 0 can start after ~1MB instead of ~3MB
                    fcuts = [0, 726, 2444, EXT]
                    nc.sync.dma_start(out=encw_sb[:, 0:1152], in_=encw_ap[:, 0:1152])
                    nc.sync.dma_start(out=feats_sb[0][:, 0:726],
                                      in_=feats_ap[0, :, 0:726])
                    nc.sync.dma_start(out=feats_sb[1][:, 0:726],
                                      in_=feats_ap[1, :, 0:726])
                    nc.sync.dma_start(out=encw_sb[:, 1152:2304],
                                      in_=encw_ap[:, 1152:2304])
                    for a, b in zip(fcuts[1:], fcuts[2:]):
                        nc.sync.dma_start(out=feats_sb[0][:, a:b],
                                          in_=feats_ap[0, :, a:b])
                        nc.sync.dma_start(out=feats_sb[1][:, a:b],
                                          in_=feats_ap[1, :, a:b])
                    # bulk / non-critical DMAs go on the scalar-issued queue so
                    # the sync queue stays clear for the collective chain
                    nc.scalar.dma_start(out=encb_sb[:], in_=encb_ap[:])
                    nc.scalar.dma_start(out=maskw_sb[:], in_=maskw_ap[:])
                    for j in range(4):
                        nc.scalar.dma_start(out=mask_sb[32 * j:32 * j + 9, :],
                                            in_=mcols_ap[j])
                    if _rep == 0:
                        nc.scalar.dma_start(out=ident_sb[:], in_=ident_ap[:])
                        nc.scalar.dma_start(out=gcnb_sb[:], in_=gcnb_ap[:])
                        nc.scalar.dma_start(out=gcnbT_sb[:], in_=gcnbT_ap[:])
                        nc.scalar.dma_start(out=rob_sb[:], in_=rob_ap[:])
                        nc.scalar.dma_start(out=gcnw12_sb[:], in_=gcnw12_ap[:])
                        nc.scalar.dma_start(out=gcnw2_sb[:], in_=gcnw2_ap[:])
                        nc.scalar.dma_start(out=row_sb[:], in_=row_ap[:])

                    # ---- enc shared feats conv (full width; the CC stream
                    # takes ~50us to come up, so a dedup AllGather here would
                    # stall longer than the 9N of duplicated PE it saves) ----
                    for nb, nw in CHUNKS:
                        ps = psp.tile([128, nw], F32, tag="cps")
                        for kt in range(2):
                            conv_mms(ps, encw_sb, kt, feats_sb[kt], nb, nw,
                                     first=(kt == 0), last=(kt == 1))
                        nc.scalar.activation(shared_sb[:, nb:nb + nw], ps[:],
                                             AF.Identity, bias=encb_sb[:])

                    # ---- per-object: mask conv + identity(shared) + relu ----
                    # (gpsimd cannot read PSUM -> scalar/vector only here)
                    ENG = ['s', 'v', 's', 'v']
                    for nb, nw in CHUNKS:
                        pss = [psp.tile([128, nw], F32, tag="cps", name=f"mps{j}")
                               for j in range(4)]
                        for j in range(4):
                            nc.tensor.matmul(pss[j][:], maskw_sb[32 * j:32 * j + 9, :],
                                             mask_sb[32 * j:32 * j + 9, nb:nb + nw],
                                             start=True, stop=False,
                                             tile_position=(32 * j, 0))
                        for j in range(4):
                            nc.tensor.matmul(pss[j][:], ident_sb[:],
                                             shared_sb[:, nb:nb + nw],
                                             start=False, stop=True)
                        for j in range(4):
                            relu_evict(cur[j], pss[j], nb, nw, ENG[j])
                            accum_lsum(j, cur, nb, nw)
                    payload_dmas(0)

                # ================= GCN x2 =================
                with tc.tile_pool(name="gcnpool", bufs=1) as gp:
                    for step in range(STEPS):
                        ar, rs_out = ar_ins[step], rs_outs[step]
                        ag_in, ag_out = ag_ins[step], ag_outs[step]
                        nc.gpsimd.collective_compute(
                            "ReduceScatter", mybir.AluOpType.add,
                            replica_groups=GROUPS,
                            ins=[ar.opt()], outs=[rs_out.opt()])
                        nc.sync.dma_start(out=tot_ext[:, 1:HEXT - 1], in_=rs_out[:])

                        # park objects 0,1,2 while the ReduceScatter flies
                        # (RS kicks right at step start; 27N of independent PE
                        # work covers its ~13us plus the late payload)
                        for j in (0, 1, 2):
                            for ci, (nb, nw) in enumerate(CHUNKS):
                                ps = psp.tile([128, nw], F32, tag="cps")
                                conv_mms(ps, gcnw12_sb, 0, cur[j], nb, nw, True, True)
                                if (ci + j) % 2 == 0:
                                    nc.scalar.activation(parks[j][:, nb:nb + nw],
                                                         ps[:], AF.Copy)
                                else:
                                    nc.vector.tensor_copy(parks[j][:, nb:nb + nw],
                                                          ps[:])
                        # half-conv of total (+gcn_b via a K=1 ones-row matmul
                        # so the eviction is a plain copy on the vector engine,
                        # keeping the scalar queue off the AllGather chain)
                        for cnb, cnw in HSUB:
                            ps = psp.tile([128, cnw], F32, tag="cps")
                            for t, (ky, kx) in enumerate(TAPS):
                                off = ky * Wp + kx
                                nc.tensor.matmul(
                                    ps[:], gcnw2_sb[:, t * 128:(t + 1) * 128],
                                    tot_ext[:, off + cnb: off + cnb + cnw],
                                    start=(t == 0), stop=False)
                            nc.tensor.matmul(ps[:], gcnbT_sb[:],
                                             ones_sb[:, 0:cnw],
                                             start=False, stop=True)
                            nc.vector.tensor_copy(hv_sb[:, cnb:cnb + cnw], ps[:])
                        # AllGather in two halves so the first quarter-rows of
                        # shared land earlier and finalize starts sooner
                        for h in range(2):
                            nc.sync.dma_start(out=ag_in[h][:],
                                              in_=hv_sb[:, h * HQ:(h + 1) * HQ])
                            nc.gpsimd.collective_compute(
                                "AllGather", mybir.AluOpType.bypass,
                                replica_groups=GROUPS,
                                ins=[ag_in[h].opt()], outs=[ag_out[h].opt()])
                        # park object 3 while the AllGather flies
                        for ci, (nb, nw) in enumerate(CHUNKS):
                            ps = psp.tile([128, nw], F32, tag="cps")
                            conv_mms(ps, gcnw12_sb, 0, cur[3], nb, nw, True, True)
                            if (ci + 1) % 2 == 0:
                                nc.scalar.activation(parks[3][:, nb:nb + nw],
                                                     ps[:], AF.Copy)
                            else:
                                nc.vector.tensor_copy(parks[3][:, nb:nb + nw], ps[:])
                        # ag half h carries local rows [16h,16h+16) of both
                        # cores: even core -> global rows 16h.., odd -> 32+16h..
                        for h in range(2):
                            nc.sync.dma_start(out=shared_sb[:, h * HQ:(h + 1) * HQ],
                                              in_=ag_out[h][0])
                            nc.sync.dma_start(
                                out=shared_sb[:, HALF + h * HQ:HALF + (h + 1) * HQ],
                                in_=ag_out[h][1])

                        # finalize: states_new = relu(park + shared)
                        dsts = [spare, cur[0], cur[1], cur[2]]
                        ENG = ['s', 'v', 's', 'v']
                        for nb, nw in CHUNKS:
                            for j in range(4):
                                nc.vector.tensor_add(parks[j][:, nb:nb + nw],
                                                     parks[j][:, nb:nb + nw],
                                                     shared_sb[:, nb:nb + nw])
                                dv = iview(dsts[j], nb, nw)
                                sv = parks[j][:, nb:nb + nw] \
                                    .rearrange("p (r u) -> p r u", u=Wp)[:, :, 1:1 + W]
                                if ENG[j] == 's':
                                    nc.scalar.activation(dv, sv, AF.Relu)
                                else:
                                    nc.vector.tensor_scalar_max(dv, sv, 0.0)
                                if step == 0:
                                    accum_lsum(j, dsts, nb, nw)
                        if step == 0:
                            payload_dmas(1)
                        new_spare = cur[3]
                        cur = [dsts[0], dsts[1], dsts[2], dsts[3]]
                        spare = new_spare

                # ================= READOUT =================
                # M=4: 54 (ktile, tap) accumulating matmuls split over 4 PE
                # column strips, issued round-robin for strip concurrency.
                with tc.tile_pool(name="ropool", bufs=1) as rp:
                    out_sb = rp.tile([4, NINT], F32, tag="outsb")
                    strips = [
                        [(4, t) for t in range(9)] + [(2, t) for t in range(5)],
                        [(5, t) for t in range(9)] + [(2, t) for t in range(5, 9)]
                        + [(3, 0)],
                        [(0, t) for t in range(4)] + [(1, t) for t in range(5)]
                        + [(3, t) for t in range(1, 5)],
                        [(0, t) for t in range(4, 9)] + [(1, t) for t in range(5, 9)]
                        + [(3, t) for t in range(5, 9)],
                    ]
                    ov = out_ap.rearrange("o (y x) -> o y x", x=W)
                    iv = out_sb[:].rearrange("o (y u) -> o y u", u=Wp)[:, :, 1:1 + W]
                    for nb, nw in CHUNKS:
                        pss = [psp.tile([128, nw], F32, tag="cps", name=f"rops{g}")
                               for g in range(4)]
                        for i in range(14):
                            for g, chain in enumerate(strips):
                                if i >= len(chain):
                                    continue
                                k, t = chain[i]
                                src = cur[k] if k < 4 else feats_sb[k - 4]
                                ky, kx = TAPS[t]
                                off = ky * Wp + kx
                                nc.tensor.matmul(
                                    pss[g][32 * g:32 * g + 4, :],
                                    row_sb[:, (k * 9 + t) * 4:(k * 9 + t + 1) * 4],
                                    src[:, off + nb: off + nb + nw],
                                    start=(i == 0), stop=(i == len(chain) - 1),
                                    tile_position=(0, 32 * g))
                        o = out_sb[:, nb:nb + nw]
                        nc.vector.tensor_copy(o, pss[0][0:4, :])
                        nc.vector.tensor_add(o, o, pss[1][32:36, :])
                        nc.vector.tensor_add(o, o, pss[2][64:68, :])
                        nc.vector.tensor_add(o, o, pss[3][96:100, :])
                        nc.scalar.activation(o, o, AF.Sigmoid, bias=rob_sb[:])
                        r0, nr = nb // Wp, nw // Wp
                        nc.scalar.dma_start(out=ov[:, r0:r0 + nr],
                                            in_=iv[:, r0:r0 + nr])

    nc.compile()
    return nc


def _host_prep(inputs):
    """Per-core input maps: shard + pad + im2col + weight lhsT layouts."""
    feats = np.asarray(inputs["batch_node_feats"], np.float32)
    masks = np.asarray(inputs["batch_previous_masks"], np.float32)
    enc_w = np.asarray(inputs["enc_w"], np.float32)
    enc_b = np.asarray(inputs["enc_b"], np.float32)
    gcn_w = np.asarray(inputs["gcn_w"], np.float32)
    gcn_b = np.asarray(inputs["gcn_b"], np.float32)
    ro_w = np.asarray(inputs["ro_w"], np.float32)
    ro_b = np.asarray(inputs["ro_b"], np.float32)

    # ---- weights (shared across cores) ----
    # enc feats part: lhsT [128cin_part, ktile, tap, cout]
    encw = enc_w[:, :C].transpose(2, 3, 1, 0).reshape(9, 2, 128, HID) \
        .transpose(2, 1, 0, 3).reshape(128, 2 * 9 * HID).copy()
    # enc mask channel: K=9 lhsT replicated at partitions {0,32,64,96}
    mvec = enc_w[:, C].transpose(1, 2, 0).reshape(9, HID)  # [tap, cout]
    maskw = np.zeros((128, 128), np.float32)
    for j in range(4):
        maskw[32 * j:32 * j + 9] = mvec
    w1 = gcn_w[:, :HID]
    w2 = gcn_w[:, HID:]
    gcnw12 = (w1 - w2).transpose(2, 3, 1, 0).reshape(9, 128, 128) \
        .transpose(1, 0, 2).reshape(128, 9 * 128).copy()
    gcnw2 = w2.transpose(2, 3, 1, 0).reshape(9, 128, 128) \
        .transpose(1, 0, 2).reshape(128, 9 * 128).copy()
    # readout: [6, 9, 128, 4]
    row = np.zeros((6, 9, 128, 4), np.float32)
    rs = ro_w[0, C:].transpose(1, 2, 0).reshape(9, HID)   # states part [tap, cin]
    for k in range(4):
        row[k, :, :, k] = rs
    for k, sl in ((4, ro_w[0, :128]), (5, ro_w[0, 128:256])):
        row[k] = sl.transpose(1, 2, 0).reshape(9, 128)[:, :, None]
    encb = enc_b.reshape(128, 1).astype(np.float32)
    gcnb = gcn_b.reshape(128, 1).astype(np.float32)
    rob = np.broadcast_to(ro_b.reshape(1, 1), (4, 1)).astype(np.float32).copy()

    in_maps = []
    for c in range(N_CORES):
        s, half = c // 2, c % 2
        # feats: pad to [C, 66, 66], flat ext [C, 4358] at offset 1
        fp = np.zeros((C, H + 2, Wp), np.float32)
        fp[:, 1:H + 1, 1:W + 1] = feats[s]
        fpflat = fp.reshape(C, PADF)
        fe = np.zeros((C, EXT), np.float32)
        fe[:, 1:1 + PADF] = fpflat
        # masks im2col: [4, 9, NINT]
        mc = np.zeros((4, 9, NINT), np.float32)
        for j in range(4):
            mp = np.zeros((H + 2, Wp), np.float32)
            mp[1:H + 1, 1:W + 1] = masks[s, 4 * half + j]
            mf = np.zeros(EXT, np.float32)
            mf[1:1 + PADF] = mp.reshape(PADF)
            for t, (ky, kx) in enumerate(TAPS):
                off = ky * Wp + kx
                mc[j, t] = mf[off:off + NINT]
        in_maps.append({
            "feats": fe.reshape(2, 128, EXT).astype(np.float16),
            "mcols": mc.astype(np.float16),
            "encw": encw.astype(np.float16), "maskw": maskw.astype(np.float16),
            "gcnw12": gcnw12.astype(np.float16), "gcnw2": gcnw2.astype(np.float16),
            "row": row.transpose(2, 0, 1, 3).reshape(128, 6 * 9 * 4).astype(np.float16),
            "ident": np.eye(128, dtype=np.float16),
            "encb": encb, "gcnb": gcnb, "rob": rob,
            "gcnbT": gcnb.reshape(1, 128).astype(np.float16),
        })
    return in_maps


def _run(inputs, repeat=1):
    from concourse.bass_utils import run_bass_kernel_spmd
    if repeat not in _PROG_CACHE:
        _PROG_CACHE[repeat] = _build_program(repeat)
    nc = _PROG_CACHE[repeat]
    in_maps = _host_prep(inputs)
    r = run_bass_kernel_spmd(nc, in_maps, list(range(N_CORES)))
    out = np.zeros((B, O, H, W), np.float32)
    for c in range(N_CORES):
        s, half = c // 2, c % 2
        out[s, 4 * half:4 * half + 4] = r.results[c]["out"].reshape(4, H, W)
    return out


def kernel(**inputs) -> np.ndarray:
    return _run(inputs, repeat=1)


# revision 40
# speedup vs baseline: 1.1024x; 1.0159x over previous
"""Bass/Trainium2 kernel for nn_GCNNTemporal (GNN message passing over object masks).

Reference computation (B=4 samples, O=8 objects, C=256, HID=128, H=W=64):
  states = relu(conv3x3(concat(feats, mask_o)))          per (sample, object)
  2x:  states_o = relu(conv3x3(concat(states_o, sum_{j!=o} states_j)))
  out_o = sigmoid(conv3x3(concat(feats, states_o)))

Sharding: 2 cores per sample, 4 objects per core. Conv3x3 SAME is 9 shifted
matmuls accumulating in PSUM over a zero-padded flat [128, 66*66] layout.

Pair-dedup of the shared convs (v2):
  - enc: conv(feats) is per-sample; each core convs only its half of the
    pixels (host supplies a haloed half `feats_half`), pairwise AllGather
    rebuilds the full encF (+enc_b folded in at eviction).
  - gcn: agg_o = total - states_o => out_o = relu(conv(s_o, w1-w2)
    + conv(total, w2) + b). ReduceScatter(padded local 4-object sums) hands
    each core its haloed half of total (rank-indexed -> SPMD parity-free);
    each core convs just that half (+gcn_b folded), AllGather rebuilds.
  - readout (Cout=1): 4 objects stacked into K with block-diagonal weights
    (M=4) over 4 PE column strips.
Chunks are row-aligned (7 rows x 66 = 462 cols); evictions write strided
views that skip the 2 pad columns per row, so pads stay zero from a single
init memset (no per-chunk cleanup).
"""
import sys
sys.path.insert(0, '/opt/trn_rl_repo')
import numpy as np

B, O, C, HID, H, W = 4, 8, 256, 128, 64, 64
STEPS = 2
N_CORES = 8

Wp = W + 2                 # padded row width (66)
PADF = (H + 2) * Wp        # 4356 padded flat size
EXT = PADF + 2             # 4358 with +-1 guard elements (data at offset 1)
NINT = H * Wp              # 4224 matmul output columns per image
INT0 = 1 + Wp              # EXT offset of interior (row 1, col 0) = 67
HALF = 32 * Wp             # 2112 cols per half (32 rows)
HEXT = 2 + 34 * Wp         # 2246: 1 guard + 34 haloed padded rows + 1 guard
CHUNKS = [(i * 7 * Wp, 7 * Wp) for i in range(9)] + [(63 * Wp, Wp)]
HSUB = [(j * 7 * Wp, 7 * Wp) for j in range(4)] + [(28 * Wp, 4 * Wp)]
TAPS = [(ky, kx) for ky in range(3) for kx in range(3)]

_PROG_CACHE = {}


def _build_program(repeat=1):
    import concourse.tile as tile
    from concourse import bacc, mybir

    AF = mybir.ActivationFunctionType
    F32 = mybir.dt.float32
    F16 = mybir.dt.float16

    nc = bacc.Bacc("TRN2", target_bir_lowering=False, debug=False,
                   num_devices=N_CORES)

    # ---- DRAM I/O ----
    feats_ap = nc.dram_tensor("feats", [2, 128, EXT], F16, kind="ExternalInput").ap()
    mcols_ap = nc.dram_tensor("mcols", [4, 9, NINT], F16, kind="ExternalInput").ap()
    encw_ap = nc.dram_tensor("encw", [128, 2 * 9 * 128], F16, kind="ExternalInput").ap()
    maskw_ap = nc.dram_tensor("maskw", [128, 128], F16, kind="ExternalInput").ap()
    gcnw12_ap = nc.dram_tensor("gcnw12", [128, 9 * 128], F16, kind="ExternalInput").ap()
    gcnw2_ap = nc.dram_tensor("gcnw2", [128, 9 * 128], F16, kind="ExternalInput").ap()
    row_ap = nc.dram_tensor("row", [128, 6 * 9 * 4], F16, kind="ExternalInput").ap()
    ident_ap = nc.dram_tensor("ident", [128, 128], F16, kind="ExternalInput").ap()
    encb_ap = nc.dram_tensor("encb", [128, 1], F32, kind="ExternalInput").ap()
    gcnb_ap = nc.dram_tensor("gcnb", [128, 1], F32, kind="ExternalInput").ap()
    gcnbT_ap = nc.dram_tensor("gcnbT", [1, 128], F16, kind="ExternalInput").ap()
    rob_ap = nc.dram_tensor("rob", [4, 1], F32, kind="ExternalInput").ap()
    out_ap = nc.dram_tensor("out", [4, H * W], F32, kind="ExternalOutput").ap()

    with tile.TileContext(nc) as tc:
        with tc.tile_pool(name="persist", bufs=1) as pp, \
             tc.tile_pool(name="psum", bufs=8, space="PSUM") as psp, \
             tc.tile_pool(name="dram", bufs=1, space="DRAM") as dp:

            # ---- persistent SBUF ----
            sts = [pp.tile([128, EXT], F16, tag=f"st{i}", name=f"st{i}")
                   for i in range(5)]
            feats_sb = [pp.tile([128, EXT], F16, tag=f"feat{k}", name=f"feat{k}")
                        for k in range(2)]
            shared_sb = pp.tile([128, NINT], F16, tag="shared")   # encF / gcn total-conv
            tot_ext = pp.tile([128, HEXT], F16, tag="totext")
            lsum_sb = pp.tile([128, NINT], F16, tag="lsum")
            hv_sb = pp.tile([128, HALF], F16, tag="hv")
            gcnw12_sb = pp.tile([128, 9 * 128], F16, tag="gw12")
            gcnw2_sb = pp.tile([128, 9 * 128], F16, tag="gw2")
            row_sb = pp.tile([128, 6 * 9 * 4], F16, tag="row")
            encb_sb = pp.tile([128, 1], F32, tag="encb")
            gcnb_sb = pp.tile([128, 1], F32, tag="gcnb")
            rob_sb = pp.tile([4, 1], F32, tag="rob")
            ident_sb = pp.tile([128, 128], F16, tag="ident")
            gcnbT_sb = pp.tile([1, 128], F16, tag="gcnbT")
            ones_sb = pp.tile([1, 7 * Wp], F16, tag="ones")
            zrow_sb = pp.tile([128, Wp], F16, tag="zrow")
            parks = [pp.tile([128, NINT], F16, tag=f"park{i}", name=f"park{i}")
                     for i in range(4)]

            # ---- DRAM collective buffers ----
            ccw_in = dp.tile([1, 1], F32, tag="ccwin")
            ccw_out = dp.tile([1, 1], F32, tag="ccwout")
            ar_ins = [dp.tile([2, 128, HALF + 2 * Wp], F16, tag=f"arin{s}",
                              name=f"arin{s}") for s in range(STEPS)]
            rs_outs = [dp.tile([128, HALF + 2 * Wp], F16, tag=f"rsout{s}",
                               name=f"rsout{s}")
                       for s in range(STEPS)]
            HQ = HALF // 2  # 1056 = 16 rows
            ag_ins = [[dp.tile([128, HQ], F16, tag=f"agin{s}{h}",
                               name=f"agin{s}{h}") for h in range(2)]
                      for s in range(STEPS)]
            ag_outs = [[dp.tile([2, 128, HQ], F16, tag=f"agout{s}{h}",
                                name=f"agout{s}{h}") for h in range(2)]
                       for s in range(STEPS)]
            GROUPS = [[0, 1], [2, 3], [4, 5], [6, 7]]

            # tiny warm-up collective absorbs CC-stream init latency
            warm_sb = pp.tile([1, 1], F32, tag="warm")
            nc.vector.memset(warm_sb[:], 0.0)
            nc.sync.dma_start(out=ccw_in[:], in_=warm_sb[:])
            nc.gpsimd.collective_compute(
                "AllReduce", mybir.AluOpType.add,
                replica_groups=GROUPS,
                ins=[ccw_in.opt()], outs=[ccw_out.opt()])

            # zero guard/pad regions once (never rewritten afterwards)
            nc.vector.memset(ones_sb[:], 1.0)
            nc.vector.memset(zrow_sb[:], 0.0)
            for t_ in sts:
                nc.vector.memset(t_[:, 0:INT0], 0.0)
                nc.vector.memset(t_[:, INT0 + NINT:EXT], 0.0)
                nc.vector.memset(t_[:, INT0:INT0 + NINT:Wp], 0.0)
                nc.vector.memset(t_[:, INT0 + Wp - 1:INT0 + NINT:Wp], 0.0)
            nc.vector.memset(tot_ext[:, 0:1], 0.0)
            nc.vector.memset(tot_ext[:, HEXT - 1:HEXT], 0.0)
            # payload shard pad rows (row above shard0 / below shard1 of the
            # padded 66-row image): zero once (scalar queue: off critical path)
            for s in range(STEPS):
                nc.scalar.dma_start(out=ar_ins[s][0, :, 0:Wp], in_=zrow_sb[:])
                nc.scalar.dma_start(out=ar_ins[s][1, :, HALF + Wp:HALF + 2 * Wp],
                                    in_=zrow_sb[:])

            def iview(t, nb, nw):
                """Strided interior view [128, nr, 64] skipping pad columns."""
                nr = nw // Wp
                return t[:, INT0 + nb:INT0 + nb + nw] \
                    .rearrange("p (r u) -> p r u", u=Wp)[:, :, 1:1 + W]

            def pview(ps, nw):
                nr = nw // Wp
                return ps[:, 0:nw].rearrange("p (r u) -> p r u", u=Wp)[:, :, 1:1 + W]

            def conv_mms(ps, w_sb, w_idx, src, nb, nw, first, last, m=128):
                """9 accumulating tap matmuls into psum ps."""
                for t, (ky, kx) in enumerate(TAPS):
                    off = ky * Wp + kx
                    nc.tensor.matmul(
                        ps[:], w_sb[:, (w_idx * 9 + t) * m:(w_idx * 9 + t + 1) * m],
                        src[:, off + nb: off + nb + nw],
                        start=(first and t == 0), stop=(last and t == 8))

            def relu_evict(dst_t, ps, nb, nw, eng):
                """dst = relu(psum), strided (pads skipped), biasless."""
                dv, sv = iview(dst_t, nb, nw), pview(ps, nw)
                if eng == 's':
                    nc.scalar.activation(dv, sv, AF.Relu)
                elif eng == 'v':
                    nc.vector.tensor_scalar_max(dv, sv, 0.0)
                else:
                    nc.gpsimd.tensor_scalar_max(dv, sv, 0.0)

            def accum_lsum(j, sts_now, nb, nw):
                """Fold object j's chunk into the running fp16 local sum."""
                if j == 0:
                    return
                acc = lsum_sb[:, nb:nb + nw]
                s = sts_now[j][:, INT0 + nb:INT0 + nb + nw]
                if j == 1:
                    nc.vector.tensor_add(acc, sts_now[0][:, INT0 + nb:INT0 + nb + nw], s)
                else:
                    nc.vector.tensor_add(acc, acc, s)

            def payload_dmas(s):
                """lsum -> padded overlapping shards of ar_ins[s]."""
                ar = ar_ins[s]
                # shard0 = padded rows 0..33: pad row (pre-zeroed) + interior
                # rows 0..32; shard1 = interior rows 31..63 + pad row.
                nc.sync.dma_start(out=ar[0, :, Wp:HALF + 2 * Wp],
                                  in_=lsum_sb[:, 0:HALF + Wp])
                nc.sync.dma_start(out=ar[1, :, 0:HALF + Wp],
                                  in_=lsum_sb[:, HALF - Wp:NINT])

            cur = [sts[0], sts[1], sts[2], sts[3]]
            spare = sts[4]

            for _rep in range(repeat):
                with tc.tile_pool(name="encpool", bufs=1) as ep:
                    encw_sb = ep.tile([128, 2 * 9 * 128], F16, tag="encw")
                    maskw_sb = ep.tile([128, 128], F16, tag="maskw")
                    mask_sb = ep.tile([128, NINT], F16, tag="maskcols")
                    # critical-path DMAs first: enc weights + feats, sliced so
                    # chunk 0 can start after ~1MB instead of ~3MB
                    fcuts = [0, 726, 2444, EXT]
                    nc.sync.dma_start(out=encw_sb[:, 0:1152], in_=encw_ap[:, 0:1152])
                    nc.sync.dma_start(out=feats_sb[0][:, 0:726],
                                      in_=feats_ap[0, :, 0:726])
                    nc.sync.dma_start(out=feats_sb[1][:, 0:726],
                                      in_=feats_ap[1, :, 0:726])
                    nc.sync.dma_start(out=encw_sb[:, 1152:2304],
                                      in_=encw_ap[:, 1152:2304])
                    for a, b in zip(fcuts[1:], fcuts[2:]):
                        nc.sync.dma_start(out=feats_sb[0][:, a:b],
                                          in_=feats_ap[0, :, a:b])
                        nc.sync.dma_start(out=feats_sb[1][:, a:b],
                                          in_=feats_ap[1, :, a:b])
                    # bulk / non-critical DMAs go on the scalar-issued queue so
                    # the sync queue stays clear for the collective chain
                    nc.scalar.dma_start(out=encb_sb[:], in_=encb_ap[:])
                    nc.scalar.dma_start(out=maskw_sb[:], in_=maskw_ap[:])
                    for j in range(4):
                        nc.scalar.dma_start(out=mask_sb[32 * j:32 * j + 9, :],
                                            in_=mcols_ap[j])
                    if _rep == 0:
                        nc.scalar.dma_start(out=ident_sb[:], in_=ident_ap[:])
                        nc.scalar.dma_start(out=gcnb_sb[:], in_=gcnb_ap[:])
                        nc.scalar.dma_start(out=gcnbT_sb[:], in_=gcnbT_ap[:])
                        nc.scalar.dma_start(out=rob_sb[:], in_=rob_ap[:])
                        nc.scalar.dma_start(out=gcnw12_sb[:], in_=gcnw12_ap[:])
                        nc.scalar.dma_start(out=gcnw2_sb[:], in_=gcnw2_ap[:])
                        nc.scalar.dma_start(out=row_sb[:], in_=row_ap[:])

                    # ---- enc shared feats conv (full width; the CC stream
                    # takes ~50us to come up, so a dedup AllGather here would
                    # stall longer than the 9N of duplicated PE it saves) ----
                    for nb, nw in CHUNKS:
                        ps = psp.tile([128, nw], F32, tag="cps")
                        for kt in range(2):
                            conv_mms(ps, encw_sb, kt, feats_sb[kt], nb, nw,
                                     first=(kt == 0), last=(kt == 1))
                        nc.scalar.activation(shared_sb[:, nb:nb + nw], ps[:],
                                             AF.Identity, bias=encb_sb[:])

                    # ---- per-object: mask conv + identity(shared) + relu ----
                    # (gpsimd cannot read PSUM -> scalar/vector only here)
                    ENG = ['s', 'v', 's', 'v']
                    for nb, nw in CHUNKS:
                        pss = [psp.tile([128, nw], F32, tag="cps", name=f"mps{j}")
                               for j in range(4)]
                        for j in range(4):
                            nc.tensor.matmul(pss[j][:], maskw_sb[32 * j:32 * j + 9, :],
                                             mask_sb[32 * j:32 * j + 9, nb:nb + nw],
                                             start=True, stop=False,
                                             tile_position=(32 * j, 0))
                        for j in range(4):
                            nc.tensor.matmul(pss[j][:], ident_sb[:],
                                             shared_sb[:, nb:nb + nw],
                                             start=False, stop=True)
                        for j in range(4):
                            relu_evict(cur[j], pss[j], nb, nw, ENG[j])
                            accum_lsum(j, cur, nb, nw)
                    payload_dmas(0)

                # ================= GCN x2 =================
                with tc.tile_pool(name="gcnpool", bufs=1) as gp:
                    for step in range(STEPS):
                        ar, rs_out = ar_ins[step], rs_outs[step]
                        ag_in, ag_out = ag_ins[step], ag_outs[step]
                        nc.gpsimd.collective_compute(
                            "ReduceScatter", mybir.AluOpType.add,
                            replica_groups=GROUPS,
                            ins=[ar.opt()], outs=[rs_out.opt()])
                        nc.sync.dma_start(out=tot_ext[:, 1:HEXT - 1], in_=rs_out[:])

                        # park objects 0,1,2 while the ReduceScatter flies
                        # (RS kicks right at step start; 27N of independent PE
                        # work covers its ~13us plus the late payload)
                        for j in (0, 1, 2):
                            for ci, (nb, nw) in enumerate(CHUNKS):
                                ps = psp.tile([128, nw], F32, tag="cps")
                                conv_mms(ps, gcnw12_sb, 0, cur[j], nb, nw, True, True)
                                if (ci + j) % 2 == 0:
                                    nc.scalar.activation(parks[j][:, nb:nb + nw],
                                                         ps[:], AF.Copy)
                                else:
                                    nc.vector.tensor_copy(parks[j][:, nb:nb + nw],
                                                          ps[:])
                        # half-conv of total (+gcn_b) over the haloed shard
                        for cnb, cnw in HSUB:
                            ps = psp.tile([128, cnw], F32, tag="cps")
                            for t, (ky, kx) in enumerate(TAPS):
                                off = ky * Wp + kx
                                nc.tensor.matmul(
                                    ps[:], gcnw2_sb[:, t * 128:(t + 1) * 128],
                                    tot_ext[:, off + cnb: off + cnb + cnw],
                                    start=(t == 0), stop=(t == 8))
                            nc.scalar.activation(hv_sb[:, cnb:cnb + cnw], ps[:],
                                                 AF.Identity, bias=gcnb_sb[:])
                        # AllGather in two halves so the first quarter-rows of
                        # shared land earlier and finalize starts sooner
                        for h in range(2):
                            nc.sync.dma_start(out=ag_in[h][:],
                                              in_=hv_sb[:, h * HQ:(h + 1) * HQ])
                            nc.gpsimd.collective_compute(
                                "AllGather", mybir.AluOpType.bypass,
                                replica_groups=GROUPS,
                                ins=[ag_in[h].opt()], outs=[ag_out[h].opt()])
                        # park object 3 while the AllGather flies
                        for ci, (nb, nw) in enumerate(CHUNKS):
                            ps = psp.tile([128, nw], F32, tag="cps")
                            conv_mms(ps, gcnw12_sb, 0, cur[3], nb, nw, True, True)
                            if (ci + 1) % 2 == 0:
                                nc.scalar.activation(parks[3][:, nb:nb + nw],
                                                     ps[:], AF.Copy)
                            else:
                                nc.vector.tensor_copy(parks[3][:, nb:nb + nw], ps[:])
                        # ag half h carries local rows [16h,16h+16) of both
                        # cores: even core -> global rows 16h.., odd -> 32+16h..
                        for h in range(2):
                            nc.sync.dma_start(out=shared_sb[:, h * HQ:(h + 1) * HQ],
                                              in_=ag_out[h][0])
                            nc.sync.dma_start(
                                out=shared_sb[:, HALF + h * HQ:HALF + (h + 1) * HQ],
                                in_=ag_out[h][1])

                        # finalize: states_new = relu(park + shared)
                        dsts = [spare, cur[0], cur[1], cur[2]]
                        ENG = ['s', 'v', 's', 'v']
                        for nb, nw in CHUNKS:
                            for j in range(4):
                                nc.vector.tensor_add(parks[j][:, nb:nb + nw],
                                                     parks[j][:, nb:nb + nw],
                                                     shared_sb[:, nb:nb + nw])
                                dv = iview(dsts[j], nb, nw)
                                sv = parks[j][:, nb:nb + nw] \
                                    .rearrange("p (r u) -> p r u", u=Wp)[:, :, 1:1 + W]
                                if ENG[j] == 's':
                                    nc.scalar.activation(dv, sv, AF.Relu)
                                else:
                                    nc.vector.tensor_scalar_max(dv, sv, 0.0)
                                if step == 0:
                                    accum_lsum(j, dsts, nb, nw)
                        if step == 0:
                            payload_dmas(1)
                        new_spare = cur[3]
                        cur = [dsts[0], dsts[1], dsts[2], dsts[3]]
                        spare = new_spare

                # ================= READOUT =================
                # M=4: 54 (ktile, tap) accumulating matmuls split over 4 PE
                # column strips, issued round-robin for strip concurrency.
                with tc.tile_pool(name="ropool", bufs=1) as rp:
                    out_sb = rp.tile([4, NINT], F32, tag="outsb")
                    strips = [
                        [(4, t) for t in range(9)] + [(2, t) for t in range(5)],
                        [(5, t) for t in range(9)] + [(2, t) for t in range(5, 9)]
                        + [(3, 0)],
                        [(0, t) for t in range(4)] + [(1, t) for t in range(5)]
                        + [(3, t) for t in range(1, 5)],
                        [(0, t) for t in range(4, 9)] + [(1, t) for t in range(5, 9)]
                        + [(3, t) for t in range(5, 9)],
                    ]
                    ov = out_ap.rearrange("o (y x) -> o y x", x=W)
                    iv = out_sb[:].rearrange("o (y u) -> o y u", u=Wp)[:, :, 1:1 + W]
                    for nb, nw in CHUNKS:
                        pss = [psp.tile([128, nw], F32, tag="cps", name=f"rops{g}")
                               for g in range(4)]
                        for i in range(14):
                            for g, chain in enumerate(strips):
                                if i >= len(chain):
                                    continue
                                k, t = chain[i]
                                src = cur[k] if k < 4 else feats_sb[k - 4]
                                ky, kx = TAPS[t]
                                off = ky * Wp + kx
                                nc.tensor.matmul(
                                    pss[g][32 * g:32 * g + 4, :],
                                    row_sb[:, (k * 9 + t) * 4:(k * 9 + t + 1) * 4],
                                    src[:, off + nb: off + nb + nw],
                                    start=(i == 0), stop=(i == len(chain) - 1),
                                    tile_position=(0, 32 * g))
                        o = out_sb[:, nb:nb + nw]
                        nc.vector.tensor_copy(o, pss[0][0:4, :])
                        nc.vector.tensor_add(o, o, pss[1][32:36, :])
                        nc.vector.tensor_add(o, o, pss[2][64:68, :])
                        nc.vector.tensor_add(o, o, pss[3][96:100, :])
                        nc.scalar.activation(o, o, AF.Sigmoid, bias=rob_sb[:])
                        r0, nr = nb // Wp, nw // Wp
                        nc.scalar.dma_start(out=ov[:, r0:r0 + nr],
                                            in_=iv[:, r0:r0 + nr])

    nc.compile()
    return nc


def _host_prep(inputs):
    """Per-core input maps: shard + pad + im2col + weight lhsT layouts."""
    feats = np.asarray(inputs["batch_node_feats"], np.float32)
    masks = np.asarray(inputs["batch_previous_masks"], np.float32)
    enc_w = np.asarray(inputs["enc_w"], np.float32)
    enc_b = np.asarray(inputs["enc_b"], np.float32)
    gcn_w = np.asarray(inputs["gcn_w"], np.float32)
    gcn_b = np.asarray(inputs["gcn_b"], np.float32)
    ro_w = np.asarray(inputs["ro_w"], np.float32)
    ro_b = np.asarray(inputs["ro_b"], np.float32)

    # ---- weights (shared across cores) ----
    # enc feats part: lhsT [128cin_part, ktile, tap, cout]
    encw = enc_w[:, :C].transpose(2, 3, 1, 0).reshape(9, 2, 128, HID) \
        .transpose(2, 1, 0, 3).reshape(128, 2 * 9 * HID).copy()
    # enc mask channel: K=9 lhsT replicated at partitions {0,32,64,96}
    mvec = enc_w[:, C].transpose(1, 2, 0).reshape(9, HID)  # [tap, cout]
    maskw = np.zeros((128, 128), np.float32)
    for j in range(4):
        maskw[32 * j:32 * j + 9] = mvec
    w1 = gcn_w[:, :HID]
    w2 = gcn_w[:, HID:]
    gcnw12 = (w1 - w2).transpose(2, 3, 1, 0).reshape(9, 128, 128) \
        .transpose(1, 0, 2).reshape(128, 9 * 128).copy()
    gcnw2 = w2.transpose(2, 3, 1, 0).reshape(9, 128, 128) \
        .transpose(1, 0, 2).reshape(128, 9 * 128).copy()
    # readout: [6, 9, 128, 4]
    row = np.zeros((6, 9, 128, 4), np.float32)
    rs = ro_w[0, C:].transpose(1, 2, 0).reshape(9, HID)   # states part [tap, cin]
    for k in range(4):
        row[k, :, :, k] = rs
    for k, sl in ((4, ro_w[0, :128]), (5, ro_w[0, 128:256])):
        row[k] = sl.transpose(1, 2, 0).reshape(9, 128)[:, :, None]
    encb = enc_b.reshape(128, 1).astype(np.float32)
    gcnb = gcn_b.reshape(128, 1).astype(np.float32)
    rob = np.broadcast_to(ro_b.reshape(1, 1), (4, 1)).astype(np.float32).copy()

    in_maps = []
    for c in range(N_CORES):
        s, half = c // 2, c % 2
        # feats: pad to [C, 66, 66], flat ext [C, 4358] at offset 1
        fp = np.zeros((C, H + 2, Wp), np.float32)
        fp[:, 1:H + 1, 1:W + 1] = feats[s]
        fpflat = fp.reshape(C, PADF)
        fe = np.zeros((C, EXT), np.float32)
        fe[:, 1:1 + PADF] = fpflat
        # masks im2col: [4, 9, NINT]
        mc = np.zeros((4, 9, NINT), np.float32)
        for j in range(4):
            mp = np.zeros((H + 2, Wp), np.float32)
            mp[1:H + 1, 1:W + 1] = masks[s, 4 * half + j]
            mf = np.zeros(EXT, np.float32)
            mf[1:1 + PADF] = mp.reshape(PADF)
            for t, (ky, kx) in enumerate(TAPS):
                off = ky * Wp + kx
                mc[j, t] = mf[off:off + NINT]
        in_maps.append({
            "feats": fe.reshape(2, 128, EXT).astype(np.float16),
            "mcols": mc.astype(np.float16),
            "encw": encw.astype(np.float16), "maskw": maskw.astype(np.float16),
            "gcnw12": gcnw12.astype(np.float16), "gcnw2": gcnw2.astype(np.float16),
            "row": row.transpose(2, 0, 1, 3).reshape(128, 6 * 9 * 4).astype(np.float16),
            "ident": np.eye(128, dtype=np.float16),
            "encb": encb, "gcnb": gcnb, "rob": rob,
            "gcnbT": gcnb.reshape(1, 128).astype(np.float16),
        })
    return in_maps


def _run(inputs, repeat=1):
    from concourse.bass_utils import run_bass_kernel_spmd
    if repeat not in _PROG_CACHE:
        _PROG_CACHE[repeat] = _build_program(repeat)
    nc = _PROG_CACHE[repeat]
    in_maps = _host_prep(inputs)
    r = run_bass_kernel_spmd(nc, in_maps, list(range(N_CORES)))
    out = np.zeros((B, O, H, W), np.float32)
    for c in range(N_CORES):
        s, half = c // 2, c % 2
        out[s, 4 * half:4 * half + 4] = r.results[c]["out"].reshape(4, H, W)
    return out


def kernel(**inputs) -> np.ndarray:
    return _run(inputs, repeat=1)
